# revision 5
# baseline (speedup 1.0000x reference)
"""Multi-head attention Bass kernel for Trainium2, 8-way sharded.

Sharding: core c handles batch b = c//2 and heads [8*(c%2), 8*(c%2)+8).
Each core computes its 8 heads' attention plus the partial output
projection over its head-slice of wo; the host sums the two partials
per batch.

All on-device matmuls run in fp16 (fp32 PSUM accumulation). The host
pre-transposes/casts operands so no transposes happen on device:
  xT  [1024, 2048]  = x[b].T
  wqT/wkT/wvT [1024, 512] = w[heads_slice, :].T
  woT [512, 1024]  = wo[:, hd_slice].T

Layout on device (per core):
  Qt/Kt [128, 4, 2048] fp16 : o-tile t holds heads 2t (partitions 0-63)
                              and 2t+1 (64-127); scoresT = Kt'. Qt' per head.
  Vext  [128, 16, 8, 65]    : per (s-tile, head) [128, 65] = [V*mask | mask];
                              column 64 yields the softmax denominator row.
  Scores are computed transposed (k on partitions), exp'd on ACT with
  scale=1/8 (softmax without max-subtraction: scores ~ N(0,1) here), then
  P_T tiles feed the PV matmul as the moving operand. Normalization uses a
  K=1 ones-broadcast matmul of the reciprocal denominator row.
"""

import numpy as np

import concourse.bass as bass  # noqa: F401  (engine namespaces live on nc)
import concourse.mybir as mybir
from concourse import bacc, bass_utils
from concourse.tile import TileContext

B, S, H = 4, 2048, 1024
HEADS, D = 16, 64
NCORES = 8
HPC = HEADS // 2  # heads per core
OS = HPC * D      # per-core head-slice width (512)

KT = H // 128     # 8  k-tiles over hidden
ST = S // 128     # 16 s-tiles
OT = OS // 128    # 4  o-tiles over the head slice (= head pairs)
QG = S // 1024    # 2  q-groups of 1024
F16 = mybir.dt.float16
F32 = mybir.dt.float32
EXP = mybir.ActivationFunctionType.Exp
ADD = mybir.AluOpType.add
MULT = mybir.AluOpType.mult

_PROGRAM = None


def _build_program():
    nc = bacc.Bacc("TRN2", target_bir_lowering=False, debug=False)

    xT_d = nc.dram_tensor("xT", [H, S], F16, kind="ExternalInput")
    wqT_d = nc.dram_tensor("wqT", [H, OS], F16, kind="ExternalInput")
    wkT_d = nc.dram_tensor("wkT", [H, OS], F16, kind="ExternalInput")
    wvT_d = nc.dram_tensor("wvT", [H, OS], F16, kind="ExternalInput")
    woT_d = nc.dram_tensor("woT", [OS, H], F16, kind="ExternalInput")
    bq_d = nc.dram_tensor("bq_r", [128, OT], F32, kind="ExternalInput")
    bk_d = nc.dram_tensor("bk_r", [128, OT], F32, kind="ExternalInput")
    bv_d = nc.dram_tensor("bv_r", [1, OS], F16, kind="ExternalInput")
    bo_d = nc.dram_tensor("bo_r", [1, H], F16, kind="ExternalInput")
    mask_d = nc.dram_tensor("mask_f", [128, ST], F32, kind="ExternalInput")
    out_d = nc.dram_tensor("out_part", [S, H], F32, kind="ExternalOutput")

    with (
        nc.allow_low_precision(reason="fp16 attention intermediates by design"),
        TileContext(nc) as tc,
    ):
        with (
            tc.tile_pool(name="persist", bufs=1) as pp,
            tc.tile_pool(name="small", bufs=1) as sp,
        ):
            Qt = pp.tile([128, OT, S], F16, tag="Qt")
            Kt = pp.tile([128, OT, S], F16, tag="Kt")
            Vext = pp.tile([128, ST, HPC, D + 1], F16, tag="Vext")
            AoT = pp.tile([128, OT, S], F16, tag="AoT")
            wo_sb = pp.tile([128, OT, H], F16, tag="wo")

            ones = sp.tile([1, 128], F16, tag="ones")
            bqr = sp.tile([128, OT], F32, tag="bqr")
            bkr = sp.tile([128, OT], F32, tag="bkr")
            bvr = sp.tile([1, OS], F16, tag="bvr")
            bor = sp.tile([1, H], F16, tag="bor")
            maskf = sp.tile([128, ST], F32, tag="maskf")

            nc.vector.memset(ones[:], 1.0)
            nc.sync.dma_start(bqr[:], bq_d.ap())
            nc.sync.dma_start(bkr[:], bk_d.ap())
            nc.sync.dma_start(bvr[:], bv_d.ap())
            nc.sync.dma_start(bor[:], bo_d.ap())
            nc.sync.dma_start(maskf[:], mask_d.ap())
            nc.sync.dma_start(
                wo_sb[:], woT_d.ap().rearrange("(k p) o -> p k o", p=128)
            )

            # ---- Stage A: Q/K/V projections -------------------------------
            with (
                tc.tile_pool(name="wx", bufs=1) as wx,
                tc.tile_pool(name="pproj", bufs=4, space="PSUM") as pj,
            ):
                xT_sb = []
                for k in range(KT):
                    t = wx.tile([128, S], F16, tag=f"xt{k}")
                    nc.sync.dma_start(t[:], xT_d.ap()[k * 128 : (k + 1) * 128, :])
                    xT_sb.append(t)
                wq_sb = wx.tile([128, KT, OS], F16, tag="wq")
                wk_sb = wx.tile([128, KT, OS], F16, tag="wk")
                wv_sb = wx.tile([128, KT, OS], F16, tag="wv")
                nc.sync.dma_start(
                    wq_sb[:], wqT_d.ap().rearrange("(k p) o -> p k o", p=128)
                )
                nc.sync.dma_start(
                    wk_sb[:], wkT_d.ap().rearrange("(k p) o -> p k o", p=128)
                )
                nc.sync.dma_start(
                    wv_sb[:], wvT_d.ap().rearrange("(k p) o -> p k o", p=128)
                )

                for ot in range(OT):
                    for sc in range(S // 512):
                        psq = pj.tile([128, 512], F32, tag="pj")
                        psk = pj.tile([128, 512], F32, tag="pj")
                        for k in range(KT):
                            nc.tensor.matmul(
                                psq[:],
                                lhsT=wq_sb[:, k, ot * 128 : (ot + 1) * 128],
                                rhs=xT_sb[k][:, sc * 512 : (sc + 1) * 512],
                                start=(k == 0),
                                stop=(k == KT - 1),
                            )
                        for k in range(KT):
                            nc.tensor.matmul(
                                psk[:],
                                lhsT=wk_sb[:, k, ot * 128 : (ot + 1) * 128],
                                rhs=xT_sb[k][:, sc * 512 : (sc + 1) * 512],
                                start=(k == 0),
                                stop=(k == KT - 1),
                            )
                        nc.vector.tensor_scalar(
                            Qt[:, ot, sc * 512 : (sc + 1) * 512],
                            psq[:], bqr[:, ot : ot + 1], None, ADD,
                        )
                        nc.vector.tensor_scalar(
                            Kt[:, ot, sc * 512 : (sc + 1) * 512],
                            psk[:], bkr[:, ot : ot + 1], None, ADD,
                        )

                for st in range(ST):
                    psv = pj.tile([128, 512], F32, tag="pj")
                    for k in range(KT):
                        nc.tensor.matmul(
                            psv[:],
                            lhsT=xT_sb[k][:, st * 128 : (st + 1) * 128],
                            rhs=wv_sb[:, k, :],
                            start=(k == 0),
                            stop=False,
                        )
                    nc.tensor.matmul(
                        psv[:], lhsT=ones[:, :], rhs=bvr[:],
                        start=False, stop=True,
                    )
                    # (V + bv) * mask, all 8 heads at once, into Vext cols 0:64
                    nc.vector.tensor_scalar(
                        Vext[:, st, :, 0:D],
                        psv[:].rearrange("p (h d) -> p h d", d=D),
                        maskf[:, st : st + 1], None, MULT,
                    )
                    # mask column (broadcast across heads)
                    nc.vector.tensor_copy(
                        Vext[:, st, :, D : D + 1],
                        maskf[:, st : st + 1, None].to_broadcast((128, HPC, 1)),
                    )

            # ---- Stage B: attention --------------------------------------
            with (
                tc.tile_pool(name="pt", bufs=6) as ptp,
                tc.tile_pool(name="rc", bufs=4) as rcp,
                tc.tile_pool(name="pscore", bufs=2, space="PSUM") as psc,
                tc.tile_pool(name="pout", bufs=4, space="PSUM") as pou,
            ):
                for t in range(OT):  # head pair
                    for g in range(QG):  # q-group of 1024
                        po = [
                            [
                                pou.tile([D + 1, 512], F32, tag="po",
                                         name=f"po_{t}_{g}_{hh}_{qc}")
                                for qc in range(2)
                            ]
                            for hh in range(2)
                        ]
                        for kt in range(ST):
                            pts = []
                            for hh in range(2):
                                r0 = hh * 64
                                ps_s = psc.tile([128, 1024], F32, tag="ps")
                                for qc in range(2):
                                    q0 = g * 1024 + qc * 512
                                    nc.tensor.matmul(
                                        ps_s[:, qc * 512 : (qc + 1) * 512],
                                        lhsT=Kt[r0 : r0 + 64, t,
                                                kt * 128 : (kt + 1) * 128],
                                        rhs=Qt[r0 : r0 + 64, t, q0 : q0 + 512],
                                        start=True, stop=True,
                                    )
                                pt_t = ptp.tile([128, 1024], F16, tag="pt")
                                nc.scalar.activation(
                                    pt_t[:], ps_s[:], EXP, scale=0.125
                                )
                                pts.append(pt_t)
                            for hh in range(2):
                                h = t * 2 + hh
                                for qc in range(2):
                                    nc.tensor.matmul(
                                        po[hh][qc][:],
                                        lhsT=Vext[:, kt, h, :],
                                        rhs=pts[hh][:, qc * 512 : (qc + 1) * 512],
                                        start=(kt == 0),
                                        stop=(kt == ST - 1),
                                    )
                        for hh in range(2):
                            for qc in range(2):
                                q0 = g * 1024 + qc * 512
                                rc16 = rcp.tile([1, 512], F16, tag="rc")
                                nc.vector.reciprocal(
                                    rc16[:], po[hh][qc][D : D + 1, :]
                                )
                                pb = psc.tile([64, 512], F32, tag="ps")
                                nc.tensor.matmul(
                                    pb[:], lhsT=ones[:, 0:64], rhs=rc16[:],
                                    start=True, stop=True,
                                )
                                # DVE has a single PSUM read port: stage the
                                # broadcast in SBUF before the multiply.
                                pb_sb = rcp.tile([64, 512], F32, tag="pbsb")
                                nc.vector.tensor_copy(pb_sb[:], pb[:])
                                nc.vector.tensor_mul(
                                    AoT[hh * 64 : hh * 64 + 64, t, q0 : q0 + 512],
                                    po[hh][qc][0:D, :], pb_sb[:],
                                )

            # ---- Stage C: output projection (partial over head slice) ----
            with (
                tc.tile_pool(name="osb", bufs=3) as osb,
                tc.tile_pool(name="pfin", bufs=4, space="PSUM") as pf,
            ):
                for st in range(ST):
                    o_sb = osb.tile([128, H], F32, tag="ot")
                    for oc in range(H // 512):
                        psf = pf.tile([128, 512], F32, tag="pf")
                        for k in range(OT):
                            nc.tensor.matmul(
                                psf[:],
                                lhsT=AoT[:, k, st * 128 : (st + 1) * 128],
                                rhs=wo_sb[:, k, oc * 512 : (oc + 1) * 512],
                                start=(k == 0),
                                stop=False,
                            )
                        nc.tensor.matmul(
                            psf[:],
                            lhsT=ones[:, :],
                            rhs=bor[:, oc * 512 : (oc + 1) * 512],
                            start=False, stop=True,
                        )
                        nc.vector.tensor_copy(
                            o_sb[:, oc * 512 : (oc + 1) * 512], psf[:]
                        )
                    nc.sync.dma_start(
                        out_d.ap()[st * 128 : (st + 1) * 128, :], o_sb[:]
                    )

    nc.compile()
    return nc


def get_program():
    global _PROGRAM
    if _PROGRAM is None:
        _PROGRAM = _build_program()
    return _PROGRAM


def make_in_maps(x, attention_mask, wq, bq, wk, bk, wv, bv, wo, bo):
    x = np.asarray(x, dtype=np.float32)
    attention_mask = np.asarray(attention_mask)
    wq, bq = np.asarray(wq, np.float32), np.asarray(bq, np.float32)
    wk, bk = np.asarray(wk, np.float32), np.asarray(bk, np.float32)
    wv, bv = np.asarray(wv, np.float32), np.asarray(bv, np.float32)
    wo, bo = np.asarray(wo, np.float32), np.asarray(bo, np.float32)

    in_maps = []
    for c in range(NCORES):
        b = c // 2
        hs = OS * (c % 2)
        sl = slice(hs, hs + OS)
        bo_c = bo if c % 2 == 0 else np.zeros_like(bo)
        in_maps.append({
            "xT": np.ascontiguousarray(x[b].T).astype(np.float16),
            "wqT": np.ascontiguousarray(wq[sl, :].T).astype(np.float16),
            "wkT": np.ascontiguousarray(wk[sl, :].T).astype(np.float16),
            "wvT": np.ascontiguousarray(wv[sl, :].T).astype(np.float16),
            "woT": np.ascontiguousarray(wo[:, sl].T).astype(np.float16),
            "bq_r": np.ascontiguousarray(bq[sl].reshape(OT, 128).T, np.float32),
            "bk_r": np.ascontiguousarray(bk[sl].reshape(OT, 128).T, np.float32),
            "bv_r": bv[sl].reshape(1, OS).astype(np.float16),
            "bo_r": bo_c.reshape(1, H).astype(np.float16),
            "mask_f": np.ascontiguousarray(
                attention_mask[b].astype(np.float32).reshape(ST, 128).T
            ),
        })
    return in_maps


def kernel(x, attention_mask, wq, bq, wk, bk, wv, bv, wo, bo, **run_kwargs):
    nc = get_program()
    in_maps = make_in_maps(x, attention_mask, wq, bq, wk, bk, wv, bv, wo, bo)
    res = bass_utils.run_bass_kernel_spmd(
        nc, in_maps, core_ids=list(range(NCORES)), **run_kwargs
    )
    parts = [r["out_part"] for r in res.results]
    out = np.stack(
        [parts[2 * b] + parts[2 * b + 1] for b in range(B)]
    ).astype(np.float32)
    kernel.last_results = res
    return out
